# revision 1
# baseline (speedup 1.0000x reference)
"""ChannelAttentionBlock Trainium2 kernel.

Computes, per batch sample (x: [B=32, C=512, H=56, W=56] fp32, gamma: [1]):
    xh = max_w(x)                  # [C, H]
    xw = max_h(x)                  # [C, W]
    w1 = channel_attn(xh); w2 = channel_attn(xw)
    out = gamma * w1[:, :, None] * x * w2[:, None, :] + x
where channel_attn(f) = softmax(rowmax(aff) - aff, axis=-1) @ f, aff = f @ f.T.

Key algebra: softmax(rowmax - aff) == softmax(-aff) row-wise (shift invariant),
so with a global stabilizer K, e = exp(K - aff) is SYMMETRIC (aff is a Gram
matrix) and attn = e / rowsum(e). Symmetry lets the stored e tiles double as
the transposed lhsT for the second matmul (no 512x512 transposes). Row sums
come free from the ACT exp's accum_out. Normalization and gamma fold into
per-channel scales applied to the tiny [C, 56] pooled outputs.

Sharding: data-parallel over batch, 4 samples per core across 8 cores.

Engine split per core: DVE does both max-pool reduces, the outer-product
build, and the fused (t+1)*x combine; ACT does exp(+rowsum) and the small
PSUM->SBUF copies/scales; PE does the matmuls/transposes. (GpSimd tensor ops
and DMA-accumulate are rejected by this container's walrus build, so the
pools stay on DVE.)
"""

import numpy as np

import concourse.bass as bass
import concourse.tile as tile
from concourse import mybir
from concourse.masks import make_identity

f32 = mybir.dt.float32
P = 128
C = 512
H = 56
W = 56
CT = C // P          # 4 c-tiles
B_TOTAL = 32
N_CORES = 8
B_PER_CORE = B_TOTAL // N_CORES   # 4

K_STAB = 280.0       # global softmax stabilizer; safe window measured [232, 331]


def _build_sample(nc, tc, pools, b, x_in, out_dram, ident, gb, kb):
    sb, ps = pools["sb"], pools["ps"]
    Exp = mybir.ActivationFunctionType.Exp

    # ---- load the 4 c-tiles of x[b] -------------------------------------
    xts = []
    for i in range(CT):
        xt = sb.tile([P, H, W], f32, tag="x", bufs=8, name=f"x_{b}_{i}")
        nc.sync.dma_start(out=xt, in_=x_in[b, i * P : (i + 1) * P, :, :])
        xts.append(xt)

    # ---- pools: xh = max over w, xw = max over h (DVE reduces) ----------
    feat_h, feat_w = [], []
    for i in range(CT):
        fh = sb.tile([P, H], f32, tag="feat", bufs=16, name=f"fh_{b}_{i}")
        nc.vector.reduce_max(out=fh, in_=xts[i], axis=mybir.AxisListType.X)
        feat_h.append(fh)

        fw = sb.tile([P, W], f32, tag="feat", bufs=16, name=f"fw_{b}_{i}")
        nc.vector.reduce_max(
            out=fw, in_=xts[i].transpose([0, 2, 1]), axis=mybir.AxisListType.X
        )
        feat_w.append(fw)

    # ---- channel attention per branch -----------------------------------
    y_scaled = []  # per branch: scaled y in PSUM (h-branch) / SBUF (w-branch)
    rr_tiles = []
    es_all = []
    for br, feats in ((0, feat_h), (1, feat_w)):
        # featT [56, 512] via 4 PE transposes into one PSUM tile + 1 copy
        tpp = ps.tile([H, CT, P], f32, tag="mm", bufs=2, name=f"tp_{b}_{br}")
        for i in range(CT):
            nc.tensor.transpose(tpp[:, i, :], feats[i], ident)
        fT = sb.tile([H, C], f32, tag="fT", bufs=4, name=f"fT_{b}_{br}")
        nc.scalar.copy(out=fT, in_=tpp)

        # aff tiles + exp(K - aff) with row-sum accumulation
        rr = sb.tile([P, CT], f32, tag="rr", bufs=4, name=f"rr_{b}_{br}")
        es = []
        for i in range(CT):
            aff = ps.tile([P, C], f32, tag="mm", bufs=2, name=f"aff_{b}_{br}_{i}")
            nc.tensor.matmul(
                aff, lhsT=fT[:, i * P : (i + 1) * P], rhs=fT, start=True, stop=True
            )
            e = sb.tile([P, C], f32, tag="e", bufs=8, name=f"e_{b}_{br}_{i}")
            nc.scalar.activation(
                out=e, in_=aff, func=Exp, bias=kb, scale=-1.0,
                accum_out=rr[:, i : i + 1],
            )
            es.append(e)
        rr_tiles.append(rr)
        es_all.append(es)

        # y[:, i, :] = sum_j e^T-chunk @ feat  (e symmetric -> stored tiles)
        y_all = ps.tile([P, CT, W], f32, tag="y", bufs=2, name=f"y_{b}_{br}")
        for i in range(CT):
            for j in range(CT):
                nc.tensor.matmul(
                    y_all[:, i, :],
                    lhsT=es[j][:, i * P : (i + 1) * P],
                    rhs=feats[j],
                    start=(j == 0),
                    stop=(j == CT - 1),
                )
        y_scaled.append(y_all)

    # ---- per-channel scales ---------------------------------------------
    # s1 = gamma / r_h   (applied to y_h, in PSUM);  s2 = 1 / r_w (into SBUF)
    rec_h = sb.tile([P, CT], f32, tag="rec", bufs=4, name=f"rech_{b}")
    nc.vector.reciprocal(out=rec_h, in_=rr_tiles[0])
    s1 = sb.tile([P, CT], f32, tag="rec", bufs=4, name=f"s1_{b}")
    nc.vector.tensor_scalar_mul(out=s1, in0=rec_h, scalar1=gb)
    rec_w = sb.tile([P, CT], f32, tag="rec", bufs=4, name=f"recw_{b}")
    nc.vector.reciprocal(out=rec_w, in_=rr_tiles[1])

    # scale y tiles on ACT (keeps DVE free): y1q = y_h * s1, y2s = y_w * rec_w
    y1q = sb.tile([P, CT, H], f32, tag="y1q", bufs=4, name=f"y1q_{b}")
    for i in range(CT):
        nc.scalar.mul(out=y1q[:, i, :], in_=y_scaled[0][:, i, :], mul=s1[:, i : i + 1])
    y2s = sb.tile([P, CT, W], f32, tag="y2s", bufs=4, name=f"y2s_{b}")
    for i in range(CT):
        nc.scalar.mul(
            out=y2s[:, i, :], in_=y_scaled[1][:, i, :], mul=rec_w[:, i : i + 1]
        )

    # ---- combine: out = (t + 1) * x, t = y1q (x) y2s outer product ------
    for i in range(CT):
        ot = sb.tile([P, H, W], f32, tag="out", bufs=2, name=f"o_{b}_{i}")
        t = sb.tile([P, H, W], f32, tag="t", bufs=2, name=f"t_{b}_{i}")
        nc.vector.tensor_mul(
            out=t,
            in0=y2s[:, i, :].unsqueeze(1).broadcast_to((P, H, W)),
            in1=y1q[:, i, :].unsqueeze(2).broadcast_to((P, H, W)),
        )
        nc.vector.scalar_tensor_tensor(
            out=ot,
            in0=t,
            scalar=1.0,
            in1=xts[i],
            op0=mybir.AluOpType.add,
            op1=mybir.AluOpType.mult,
        )
        nc.sync.dma_start(out=out_dram[b, i * P : (i + 1) * P, :, :], in_=ot)


def _build():
    nc = bass.Bass()
    x_in = nc.dram_tensor("x", [B_PER_CORE, C, H, W], f32, kind="ExternalInput")
    g_in = nc.dram_tensor("gamma", [1], f32, kind="ExternalInput")
    out_dram = nc.dram_tensor(
        "out", [B_PER_CORE, C, H, W], f32, kind="ExternalOutput"
    )

    with tile.TileContext(nc) as tc:
        with (
            tc.tile_pool(name="consts", bufs=1) as consts,
            tc.tile_pool(name="sb", bufs=2) as sb,
            tc.tile_pool(name="ps", bufs=1, space="PSUM") as ps,
        ):
            ident = consts.tile([P, P], f32, tag="id", name="ident")
            make_identity(nc, ident)
            gb = consts.tile([P, 1], f32, tag="gb", name="gb")
            nc.sync.dma_start(out=gb, in_=g_in[:].to_broadcast((P, 1)))
            kb = consts.tile([P, 1], f32, tag="kb", name="kb")
            nc.vector.memset(kb, K_STAB)

            pools = {"sb": sb, "ps": ps}
            for b in range(B_PER_CORE):
                _build_sample(nc, tc, pools, b, x_in, out_dram, ident, gb, kb)
    return nc


def _split_attached_waits(raw: bytes) -> bytes:
    """Move every attached on_wait into a standalone EventSemaphore instruction
    placed directly before its owner (same engine stream, same semantics: the
    sequencer blocks, then dispatches the op). The walrus build in this
    environment rejects instructions whose EVENTS struct carries more sync-wait
    commands than it has slots; standalone one-wait EventSemaphore instructions
    are the raw-bass style it always accepts."""
    import json

    bir = json.loads(raw)
    for fn in bir["functions"]:
        for blk in fn["blocks"]:
            new = []
            for inst in blk["instructions"]:
                si = inst.get("sync_info")
                ow = (si or {}).get("on_wait") or []
                if ow and inst.get("opcode") != "EventSemaphore":
                    for k, w in enumerate(ow):
                        new.append(
                            {
                                "debug": inst.get("debug", 0),
                                "engine": inst["engine"],
                                "ins": [],
                                "outs": [],
                                "name": f"{inst['name']}_sw{k}",
                                "opcode": "EventSemaphore",
                                "sync_info": {"on_update": [], "on_wait": [w]},
                            }
                        )
                    si["on_wait"] = []
                new.append(inst)
            blk["instructions"] = new
    return json.dumps(bir).encode()


_NC_CACHE = None


def _get_nc():
    global _NC_CACHE
    if _NC_CACHE is None:
        nc = _build()
        orig = nc.to_json_bytes
        nc.to_json_bytes = lambda: _split_attached_waits(orig())
        _NC_CACHE = nc
    return _NC_CACHE


def kernel(x, gamma):
    from concourse.bass_utils import run_bass_kernel_spmd

    x = np.ascontiguousarray(np.asarray(x), dtype=np.float32)
    gamma = np.ascontiguousarray(np.asarray(gamma), dtype=np.float32)
    nc = _get_nc()
    in_maps = [
        {"x": x[c * B_PER_CORE : (c + 1) * B_PER_CORE], "gamma": gamma}
        for c in range(N_CORES)
    ]
    res = run_bass_kernel_spmd(nc, in_maps, core_ids=list(range(N_CORES)))
    return np.concatenate([r["out"] for r in res.results], axis=0)



# revision 11
# speedup vs baseline: 1.3843x; 1.3843x over previous
"""ChannelAttentionBlock Trainium2 kernel (fp16/bf16, software-pipelined).

Computes, per batch sample (x: [B=32, C=512, H=56, W=56] fp32, gamma: [1]):
    xh = max_w(x)                  # [C, H]
    xw = max_h(x)                  # [C, W]
    w1 = channel_attn(xh); w2 = channel_attn(xw)
    out = gamma * w1[:, :, None] * x * w2[:, None, :] + x
where channel_attn(f) = softmax(rowmax(aff) - aff, axis=-1) @ f, aff = f @ f.T.

Key algebra: softmax(rowmax - aff) == softmax(-aff) row-wise (shift invariant),
so with a global stabilizer K, e = exp(K - aff) is SYMMETRIC (aff is a Gram
matrix) and attn = e / rowsum(e). Symmetry lets the stored e tiles double as
the transposed lhsT for the second matmul (no 512x512 transposes). Row sums
come free from the ACT exp's accum_out.

Precision plan (rel-err gate is 2e-2; measured ~3.3e-3 end to end in numpy):
  - x converted once to fp16 on ACT; pools, outer product and the final
    combine all read the fp16 copy (fp16 rounding is 5e-4 relative).
  - Both max-pools run on DVE as fp16 tensor-max trees over contiguous
    halves: every level is a packed 2-byte TensorTensor, which the DVE
    executes in 2x mode, and neither pool needs strided/transposed reads.
  - aff stays f32 in PSUM (fp16 feats keep its absolute error ~0.02, which
    exp() tolerates); e = exp(K - aff) is stored bf16 (needs f32-like
    exponent range), row sums accumulate in f32.
  - Matmuls run fp16/bf16 on PE (1 cycle/row vs 4 for fp32).
  - 1/rowsum is computed on ACT as Exp(-Ln(r)) (the direct Reciprocal
    activation is blocked for accuracy; DVE reciprocal would park the whole
    scale chain behind the next sample's pool trees in DVE's in-order
    stream). gamma folds into the Exp bias as ln(gamma).
  - Combine: ACT replicates y1q along w into the t buffer, DVE multiplies
    in-place by y2s broadcast along h (packed fp16 2x), then one f32
    scalar_tensor_tensor per c-tile does out = (t + 1) * x16.

Software pipeline: emission order IS each engine's execution order, so
stages are emitted skewed (H0 H1 M0 T0 H2 M1 T1 H3 M2 T2 M3 T3) to keep
DMA/ACT/DVE streaming on independent samples. Engine budget per core (cost
model): DVE ~140us, DMA 143us, ACT ~110us, PE ~20us.

Sharding: data-parallel over batch, 4 samples per core across 8 cores.
"""

import numpy as np

import concourse.bass as bass
import concourse.tile as tile
from concourse import mybir
from concourse.masks import make_identity

f32 = mybir.dt.float32
f16 = mybir.dt.float16
bf16 = mybir.dt.bfloat16
P = 128
C = 512
H = 56
W = 56
CT = C // P          # 4 c-tiles
NP = CT // 2         # 2 c-tile pairs
B_TOTAL = 32
N_CORES = 8
B_PER_CORE = B_TOTAL // N_CORES   # 4

K_STAB = 280.0       # global softmax stabilizer; safe window measured [232, 331]
Exp = mybir.ActivationFunctionType.Exp
Ln = mybir.ActivationFunctionType.Ln


def _head(nc, sb, b, j, x_in, x16s, fh16, fw16):
    """Per c-tile pair: DMA loads, fp16 convert, both max-pool trees."""
    x16 = sb.tile([P, 2, H, W], f16, tag="x16", bufs=5, name=f"x16_{b}_{j}")
    for k in range(2):
        i = 2 * j + k
        x32 = sb.tile([P, H, W], f32, tag="x32", bufs=3, name=f"x32_{b}_{i}")
        nc.sync.dma_start(out=x32, in_=x_in[b, i * P : (i + 1) * P, :, :])
        nc.scalar.copy(out=x16[:, k], in_=x32)
    x16s[(b, j)] = x16

    # W-pool: fh[c, h] = max_w x[c, h, w]
    w1 = sb.tile([P, 2, H, 28], f16, tag="w1", bufs=1, name=f"w1_{b}_{j}")
    nc.vector.tensor_max(out=w1, in0=x16[:, :, :, 0:28], in1=x16[:, :, :, 28:56])
    w2 = sb.tile([P, 2, H, 14], f16, tag="w2", bufs=1, name=f"w2_{b}_{j}")
    nc.vector.tensor_max(out=w2, in0=w1[:, :, :, 0:14], in1=w1[:, :, :, 14:28])
    w3 = sb.tile([P, 2, H, 7], f16, tag="w3", bufs=1, name=f"w3_{b}_{j}")
    nc.vector.tensor_max(out=w3, in0=w2[:, :, :, 0:7], in1=w2[:, :, :, 7:14])
    w4 = sb.tile([P, 2, H, 4], f16, tag="w4", bufs=1, name=f"w4_{b}_{j}")
    nc.vector.tensor_max(out=w4, in0=w3[:, :, :, 0:4], in1=w3[:, :, :, 3:7])
    w5 = sb.tile([P, 2, H, 2], f16, tag="w5", bufs=1, name=f"w5_{b}_{j}")
    nc.vector.tensor_max(out=w5, in0=w4[:, :, :, 0:2], in1=w4[:, :, :, 2:4])
    nc.vector.tensor_max(
        out=fh16[:, 2 * j : 2 * j + 2, :].unsqueeze(3),
        in0=w5[:, :, :, 0:1],
        in1=w5[:, :, :, 1:2],
    )
    # H-pool: fw[c, w] = max_h x[c, h, w] (contiguous row halves, no strides)
    h1 = sb.tile([P, 2, 28, W], f16, tag="h1", bufs=1, name=f"h1_{b}_{j}")
    nc.vector.tensor_max(out=h1, in0=x16[:, :, 0:28, :], in1=x16[:, :, 28:56, :])
    h2 = sb.tile([P, 2, 14, W], f16, tag="h2", bufs=1, name=f"h2_{b}_{j}")
    nc.vector.tensor_max(out=h2, in0=h1[:, :, 0:14, :], in1=h1[:, :, 14:28, :])
    h3 = sb.tile([P, 2, 7, W], f16, tag="h3", bufs=1, name=f"h3_{b}_{j}")
    nc.vector.tensor_max(out=h3, in0=h2[:, :, 0:7, :], in1=h2[:, :, 7:14, :])
    h4 = sb.tile([P, 2, 4, W], f16, tag="h4", bufs=1, name=f"h4_{b}_{j}")
    nc.vector.tensor_max(out=h4, in0=h3[:, :, 0:4, :], in1=h3[:, :, 3:7, :])
    h5 = sb.tile([P, 2, 2, W], f16, tag="h5", bufs=1, name=f"h5_{b}_{j}")
    nc.vector.tensor_max(out=h5, in0=h4[:, :, 0:2, :], in1=h4[:, :, 2:4, :])
    nc.vector.tensor_max(
        out=fw16[:, 2 * j : 2 * j + 2, :].unsqueeze(2),
        in0=h5[:, :, 0:1, :],
        in1=h5[:, :, 1:2, :],
    )


def _mid(nc, sb, ps, b, fh16, fw16, ident16, lgb, kb):
    """Per sample: CxC channel attention on the pooled [C, 56] features for
    both branches; returns the scaled attention outputs y1q, y2s (f16)."""
    y_scaled = []
    for br, feats in ((0, fh16), (1, fw16)):
        # featT [56, 512] f16 via 4 PE transposes into one PSUM tile + 1 copy
        tpp = ps.tile([H, CT, P], f16, tag="tp", bufs=2, name=f"tp_{b}_{br}")
        for i in range(CT):
            nc.tensor.transpose(tpp[:, i, :], feats[:, i, :], ident16)
        fT = sb.tile([H, C], f16, tag="fT", bufs=4, name=f"fT_{b}_{br}")
        nc.scalar.copy(out=fT, in_=tpp)

        # aff tiles (f32 PSUM) + e = exp(K - aff) (bf16) with f32 row sums
        rr = sb.tile([P, CT], f32, tag="rr", bufs=4, name=f"rr_{b}_{br}")
        es = []
        for i in range(CT):
            aff = ps.tile([P, C], f32, tag="aff", bufs=4, name=f"aff_{b}_{br}_{i}")
            nc.tensor.matmul(
                aff, lhsT=fT[:, i * P : (i + 1) * P], rhs=fT, start=True, stop=True
            )
            e = sb.tile([P, C], bf16, tag="e", bufs=8, name=f"e_{b}_{br}_{i}")
            nc.scalar.activation(
                out=e, in_=aff, func=Exp, bias=kb, scale=-1.0,
                accum_out=rr[:, i : i + 1],
            )
            es.append(e)

        # y[:, i, :] = sum_j e^T-chunk @ feat  (e symmetric -> stored tiles)
        y_all = ps.tile([P, CT, W], f32, tag="y", bufs=2, name=f"y_{b}_{br}")
        for i in range(CT):
            for j in range(CT):
                nc.tensor.matmul(
                    y_all[:, i, :],
                    lhsT=es[j][:, i * P : (i + 1) * P],
                    rhs=feats[:, j, :],
                    start=(j == 0),
                    stop=(j == CT - 1),
                )

        # scale = gamma / r_h (h branch), 1 / r_w (w branch), all on ACT:
        # 1/r as Exp(-Ln(r)) with ln(gamma) folded into the Exp bias.
        lnr = sb.tile([P, CT], f32, tag="rec", bufs=8, name=f"lnr_{b}_{br}")
        nc.scalar.activation(out=lnr, in_=rr, func=Ln)
        rec = sb.tile([P, CT], f32, tag="rec", bufs=8, name=f"rec_{b}_{br}")
        nc.scalar.activation(
            out=rec, in_=lnr, func=Exp, scale=-1.0, bias=(lgb if br == 0 else 0.0)
        )
        yq = sb.tile([P, CT, W], f16, tag="yq", bufs=4, name=f"yq_{b}_{br}")
        for i in range(CT):
            nc.scalar.mul(out=yq[:, i, :], in_=y_all[:, i, :], mul=rec[:, i : i + 1])
        y_scaled.append(yq)
    return y_scaled


def _tail(nc, sb, b, j, y1q, y2s, x16s, out_dram):
    """Per c-tile pair: out = (t + 1) * x16, t = y1q (x) y2s outer product.
    ACT replicates y1q along w into t; DVE multiplies in place by y2s
    broadcast along h (both operands packed fp16 -> 2x mode)."""
    x16 = x16s.pop((b, j))
    t16 = sb.tile([P, 2, H, W], f16, tag="t16", bufs=2, name=f"t16_{b}_{j}")
    nc.scalar.copy(
        out=t16,
        in_=y1q[:, 2 * j : 2 * j + 2, :].unsqueeze(3).broadcast_to((P, 2, H, W)),
    )
    nc.vector.tensor_mul(
        out=t16,
        in0=t16,
        in1=y2s[:, 2 * j : 2 * j + 2, :].unsqueeze(2).broadcast_to((P, 2, H, W)),
    )
    for k in range(2):
        i = 2 * j + k
        ot = sb.tile([P, H, W], f32, tag="out", bufs=3, name=f"o_{b}_{i}")
        nc.vector.scalar_tensor_tensor(
            out=ot,
            in0=t16[:, k],
            scalar=1.0,
            in1=x16[:, k],
            op0=mybir.AluOpType.add,
            op1=mybir.AluOpType.mult,
        )
        # store issued from the idle Pool engine's SWDGE queue so it never
        # blocks the SP queue's loads (head-of-line) behind an unfinished
        # compute.
        nc.gpsimd.dma_start(out=out_dram[b, i * P : (i + 1) * P, :, :], in_=ot)


def _build_pipelined(nc, tc, sb, ps, x_in, out_dram, ident16, lgb, kb):
    x16s = {}
    feats = {}
    ys = {}

    def head_pair(b, j):
        if j == 0:
            fh16 = sb.tile([P, CT, H], f16, tag="feat", bufs=4, name=f"fh_{b}")
            fw16 = sb.tile([P, CT, W], f16, tag="feat", bufs=4, name=f"fw_{b}")
            feats[b] = (fh16, fw16)
        fh16, fw16 = feats[b]
        _head(nc, sb, b, j, x_in, x16s, fh16, fw16)

    def mid(b):
        fh16, fw16 = feats.pop(b)
        ys[b] = _mid(nc, sb, ps, b, fh16, fw16, ident16, lgb, kb)

    def tail_pair(b, j):
        y1q, y2s = ys[b]
        _tail(nc, sb, b, j, y1q, y2s, x16s, out_dram)

    # skewed schedule with tail pairs interleaved into the next head:
    # H0 H1 M0 [T0p0 H2p0 T0p1 H2p1] M1 [T1p0 H3p0 T1p1 H3p1] M2 T2 M3 T3
    B = B_PER_CORE
    for j in range(NP):
        head_pair(0, j)
    for j in range(NP):
        head_pair(1, j)
    for b in range(B):
        mid(b)
        for j in range(NP):
            tail_pair(b, j)
            if b + 2 < B:
                head_pair(b + 2, j)


def _build():
    nc = bass.Bass()
    x_in = nc.dram_tensor("x", [B_PER_CORE, C, H, W], f32, kind="ExternalInput")
    g_in = nc.dram_tensor("gamma", [1], f32, kind="ExternalInput")
    out_dram = nc.dram_tensor(
        "out", [B_PER_CORE, C, H, W], f32, kind="ExternalOutput"
    )

    with tile.TileContext(nc) as tc:
        with (
            tc.tile_pool(name="consts", bufs=1) as consts,
            tc.tile_pool(name="sb", bufs=2) as sb,
            tc.tile_pool(name="ps", bufs=1, space="PSUM") as ps,
        ):
            ident16 = consts.tile([P, P], f16, tag="id", name="ident16")
            make_identity(nc, ident16)
            gb = consts.tile([P, 1], f32, tag="gb", name="gb")
            nc.sync.dma_start(out=gb, in_=g_in[:].to_broadcast((P, 1)))
            lgb = consts.tile([P, 1], f32, tag="lgb", name="lgb")
            nc.scalar.activation(out=lgb, in_=gb, func=Ln)
            kb = consts.tile([P, 1], f32, tag="kb", name="kb")
            nc.vector.memset(kb, K_STAB)

            _build_pipelined(nc, tc, sb, ps, x_in, out_dram, ident16, lgb, kb)
    return nc


def _split_attached_waits(raw: bytes) -> bytes:
    """Move every attached on_wait into a standalone EventSemaphore instruction
    placed directly before its owner (same engine stream, same semantics: the
    sequencer blocks, then dispatches the op). The walrus build in this
    environment rejects instructions whose EVENTS struct carries more sync-wait
    commands than it has slots; standalone one-wait EventSemaphore instructions
    are the raw-bass style it always accepts."""
    import json

    bir = json.loads(raw)
    for fn in bir["functions"]:
        for blk in fn["blocks"]:
            new = []
            for inst in blk["instructions"]:
                si = inst.get("sync_info")
                ow = (si or {}).get("on_wait") or []
                if ow and inst.get("opcode") != "EventSemaphore":
                    for k, w in enumerate(ow):
                        new.append(
                            {
                                "debug": inst.get("debug", 0),
                                "engine": inst["engine"],
                                "ins": [],
                                "outs": [],
                                "name": f"{inst['name']}_sw{k}",
                                "opcode": "EventSemaphore",
                                "sync_info": {"on_update": [], "on_wait": [w]},
                            }
                        )
                    si["on_wait"] = []
                new.append(inst)
            blk["instructions"] = new
    return json.dumps(bir).encode()


_NC_CACHE = None


def _get_nc():
    global _NC_CACHE
    if _NC_CACHE is None:
        nc = _build()
        orig = nc.to_json_bytes
        nc.to_json_bytes = lambda: _split_attached_waits(orig())
        _NC_CACHE = nc
    return _NC_CACHE


def kernel(x, gamma):
    from concourse.bass_utils import run_bass_kernel_spmd

    x = np.ascontiguousarray(np.asarray(x), dtype=np.float32)
    gamma = np.ascontiguousarray(np.asarray(gamma), dtype=np.float32)
    nc = _get_nc()
    in_maps = [
        {"x": x[c * B_PER_CORE : (c + 1) * B_PER_CORE], "gamma": gamma}
        for c in range(N_CORES)
    ]
    res = run_bass_kernel_spmd(nc, in_maps, core_ids=list(range(N_CORES)))
    return np.concatenate([r["out"] for r in res.results], axis=0)
